# revision 1
# baseline (speedup 1.0000x reference)
"""Trainium2 Bass kernel v2 for the GRU decoder (nn_Decoder).

Same host-side algebraic folding as the baseline (h-only recurrence with
W_eff = W_ih @ lin_w; vocab projection W_big = linres_w @ lin_w; step 0 on
host).  Device-side restructure:

  - per-step hidden state kept as ONE [128, KCH*128] bf16 tile per step
    (transposed chunk layout: column k*128+b holds h[b, k*128+p]); it is the
    stationary operand of both the gate matmuls and the vocab projection.
  - the h^T tile is produced by 8 PE transposes (identity matmul) into one
    PSUM bank + one Act copy -- no DMA transposes on the critical path.
  - single-phase schedule: the 13 projection chunks of step t-1 are emitted
    inside step t's PE window (after the gate matmuls, split around the
    transposes), so the PE never idles and stays out of p-state ramps.
  - gate/projection biases are added by DVE in-place in PSUM (gates) or
    fused into the psum->bf16 output conversion (projection); sigmoid/tanh/
    relu on Act; h-update tail (d/e/h) on GpSimd.  All matmuls bf16 with
    512-wide moving streams (same shapes as the validated baseline).
  - logits are written as bf16 (tolerance is 2e-2; bf16 adds ~2e-3), in
    [128, 2048]-column groups via the otherwise-idle SP engine's queue.
"""

import numpy as np
import ml_dtypes

import concourse.bass as bass
import concourse.tile as tile
from concourse import mybir
from concourse.bass_utils import run_bass_kernel_spmd

F32 = mybir.dt.float32
BF16 = mybir.dt.bfloat16
AF = mybir.ActivationFunctionType

B = 128
H = 1024
V = 50257
KCH = H // 128
N_CORES = 8
VS = 6283  # per-core vocab slice (8*6283 = 50264 >= V; tail zero-padded)
NCH = (VS + 511) // 512  # 13 projection chunks (last one 139 wide)
OGW = 1024  # output DMA column-group width

_bf16 = ml_dtypes.bfloat16


def patch_excess_waits(nc, maxw=1):
    """The walrus build in this container rejects >1 sync-wait attached to a
    single instruction; hoist extras into standalone EventSemaphore insts."""
    for fn in nc.m.functions:
        for bb in fn.blocks:
            new_insts = []
            for inst in bb.instructions:
                si = getattr(inst, "sync_info", None)
                if si and si.on_wait and len(si.on_wait) > maxw:
                    waits = list(si.on_wait)
                    excess, keep = waits[:-maxw], waits[-maxw:]
                    for w in excess:
                        new_insts.append(
                            mybir.InstEventSemaphore(
                                name=nc.get_next_instruction_name(),
                                opcode="EventSemaphore",
                                engine=inst.engine,
                                ins=[],
                                outs=[],
                                sync_info=mybir.SyncInfo(on_wait=[w], on_update=[]),
                            )
                        )
                    si.on_wait = keep
                new_insts.append(inst)
            bb.instructions[:] = new_insts


def build_program(T=30, n_reps=0, patch=True):
    nc = bass.Bass("TRN2", target_bir_lowering=False, debug=False)

    wcat_ap = nc.dram_tensor("wcat", [KCH, 128, 4096], BF16, kind="ExternalInput").ap()
    gbias_ap = nc.dram_tensor("gbias", [128, 4096], BF16, kind="ExternalInput").ap()
    h1s_ap = nc.dram_tensor("h1s", [128, H], BF16, kind="ExternalInput").ap()
    h1b_ap = nc.dram_tensor("h1b", [B, H], BF16, kind="ExternalInput").ap()
    ident_ap = nc.dram_tensor("ident", [128, 128], BF16, kind="ExternalInput").ap()
    wproj_ap = nc.dram_tensor("wproj", [KCH, 128, VS], BF16, kind="ExternalInput").ap()
    bproj_ap = nc.dram_tensor("bproj", [128, VS], BF16, kind="ExternalInput").ap()
    out_ap = nc.dram_tensor("out", [T * 128, VS], BF16, kind="ExternalOutput").ap()

    def emit_proj(tc, m, c_lo, c_hi, hp, wp, bpr, obs, prp, psp):
        """Projection chunks [c_lo, c_hi) of m-block m from hstate tile hp.
        obs: dict carrying the current output-group tile across calls."""
        nc = tc.nc
        for c in range(c_lo, c_hi):
            cw = min(512, VS - c * 512)
            co = c * 512
            g = co // OGW
            off = co - g * OGW
            gw = min(OGW, VS - g * OGW)
            if off == 0:
                ob_new = prp.tile([128, OGW], BF16, tag="ob")
                obs[m] = ob_new
            ob = obs[m]
            pp = psp.tile([128, 512], F32, tag="pps")
            for k in range(KCH):
                nc.tensor.matmul(
                    pp[:, :cw],
                    hp[:, k * 128 : (k + 1) * 128],
                    wp[k][:, co : co + cw],
                    start=(k == 0),
                    stop=(k == KCH - 1),
                )
            nc.vector.tensor_add(ob[:, off : off + cw], pp[:, :cw], bpr[:, co : co + cw])
            if off + cw >= gw:  # group complete -> relu + store
                nc.scalar.activation(ob[:, :gw], ob[:, :gw], AF.Relu)
                nc.sync.dma_start(
                    out_ap[m * 128 : (m + 1) * 128, g * OGW : g * OGW + gw],
                    ob[:, :gw],
                )

    def body(tc):
        nc = tc.nc
        with (
            tc.tile_pool(name="wg", bufs=1) as wgp,
            tc.tile_pool(name="wp", bufs=1) as wpp,
            tc.tile_pool(name="hs", bufs=3) as hsp,
            tc.tile_pool(name="hb", bufs=2) as hbp,
            tc.tile_pool(name="ew", bufs=1) as ewp,
            tc.tile_pool(name="obp", bufs=2) as prp,
            tc.tile_pool(name="gps", bufs=4, space="PSUM") as psg,
            tc.tile_pool(name="pps", bufs=3, space="PSUM") as psp,
            tc.tile_pool(name="tps", bufs=1, space="PSUM") as ptp,
        ):
            # ---- resident weights ----
            wg = []
            for k in range(KCH):
                t = wgp.tile([128, 4096], BF16, name=f"wg{k}")
                nc.sync.dma_start(t[:], wcat_ap[k])
                wg.append(t)
            gbias = wgp.tile([128, 4096], BF16, name="gbias")
            nc.sync.dma_start(gbias[:], gbias_ap[:])
            ident = wpp.tile([128, 128], BF16, name="ident")
            nc.sync.dma_start(ident[:], ident_ap[:])

            h_prev = hsp.tile([128, H], BF16, tag="hs")
            nc.sync.dma_start(h_prev[:], h1s_ap[:])
            hb_prev = hbp.tile([B, H], BF16, tag="hb")
            nc.sync.dma_start(hb_prev[:], h1b_ap[:])

            wp = [wpp.tile([128, VS], BF16, name=f"wp{k}") for k in range(KCH)]
            # column-major-ish loads so early projection chunks are ready fast
            for cg in range(0, VS, 2048):
                ce = min(VS, cg + 2048)
                for k in range(KCH):
                    nc.sync.dma_start(wp[k][:, cg:ce], wproj_ap[k][:, cg:ce])
            bpr = wpp.tile([128, VS], BF16, name="bpr")
            nc.sync.dma_start(bpr[:], bproj_ap[:])

            # persistent elementwise work tiles
            rz = ewp.tile([128, 2048], BF16, name="rz")
            t1 = ewp.tile([128, 1024], BF16, name="t1")
            nt = ewp.tile([128, 1024], BF16, name="nt")
            dd = ewp.tile([128, 1024], BF16, name="dd")

            obs = {}
            for s in range(1, T):
                # ---- gates for step s (PE) + elementwise as groups finish ----
                # groups: rz (cols 0:2048), hn (3072:4096), in (2048:3072)
                gp = {}
                for ci, c0 in enumerate((0, 512, 1024, 1536)):
                    p = psg.tile([B, 512], F32, tag="g")
                    for k in range(KCH):
                        nc.tensor.matmul(
                            p[:],
                            h_prev[:, k * 128 : (k + 1) * 128],
                            wg[k][:, c0 : c0 + 512],
                            start=(k == 0),
                            stop=(k == KCH - 1),
                        )
                    nc.vector.tensor_add(p[:], p[:], gbias[:, c0 : c0 + 512])
                    nc.scalar.activation(rz[:, c0 : c0 + 512], p[:], AF.Sigmoid)
                for ci, c0 in enumerate((3072, 3584)):
                    p = psg.tile([B, 512], F32, tag="g")
                    for k in range(KCH):
                        nc.tensor.matmul(
                            p[:],
                            h_prev[:, k * 128 : (k + 1) * 128],
                            wg[k][:, c0 : c0 + 512],
                            start=(k == 0),
                            stop=(k == KCH - 1),
                        )
                    jj = c0 - 3072
                    nc.vector.tensor_add(p[:], p[:], gbias[:, c0 : c0 + 512])
                    nc.vector.tensor_mul(
                        t1[:, jj : jj + 512], rz[:, jj : jj + 512], p[:]
                    )
                for ci, c0 in enumerate((2048, 2560)):
                    p = psg.tile([B, 512], F32, tag="g")
                    for k in range(KCH):
                        nc.tensor.matmul(
                            p[:],
                            h_prev[:, k * 128 : (k + 1) * 128],
                            wg[k][:, c0 : c0 + 512],
                            start=(k == 0),
                            stop=(k == KCH - 1),
                        )
                    jj = c0 - 2048
                    nc.vector.tensor_add(p[:], p[:], gbias[:, c0 : c0 + 512])
                    nc.vector.tensor_add(p[:], p[:], t1[:, jj : jj + 512])
                    nc.scalar.activation(nt[:, jj : jj + 512], p[:], AF.Tanh)
                # h = n + z*(h_prev - n)   (GpSimd, sbuf-only)
                nc.gpsimd.tensor_sub(dd[:], hb_prev[:], nt[:])
                nc.gpsimd.tensor_mul(dd[:], rz[:, 1024:2048], dd[:])
                hb_cur = hbp.tile([B, H], BF16, tag="hb")
                nc.gpsimd.tensor_add(hb_cur[:], nt[:], dd[:])

                # ---- projection of m = s-1, first chunks (PE keeps busy) ----
                emit_proj(tc, s - 1, 0, 8, h_prev, wp, bpr, obs, prp, psp)

                # ---- PE transposes of h_bf -> psum -> hstate tile ----
                ptr = ptp.tile([128, H], BF16, tag="tr")
                for k in range(KCH):
                    nc.tensor.transpose(
                        ptr[:, k * 128 : (k + 1) * 128],
                        hb_cur[:, k * 128 : (k + 1) * 128],
                        ident[:],
                    )
                h_cur = hsp.tile([128, H], BF16, tag="hs")
                nc.scalar.copy(h_cur[:], ptr[:])

                # ---- projection of m = s-1, remaining chunks ----
                emit_proj(tc, s - 1, 8, NCH, h_prev, wp, bpr, obs, prp, psp)

                h_prev = h_cur
                hb_prev = hb_cur

            # last m-block
            emit_proj(tc, T - 1, 0, NCH, h_prev, wp, bpr, obs, prp, psp)

    with tile.TileContext(nc) as tc:
        if n_reps > 0:
            with tc.For_i(0, n_reps, 1):
                body(tc)
        else:
            body(tc)

    if patch:
        patch_excess_waits(nc)
    return nc


# ---------------- host side ----------------


def fold_weights(W_ih, W_hh, b_ih, b_hh, lin_w, lin_b, linres_w, linres_b):
    W_ih = W_ih.astype(np.float64)
    W_hh = W_hh.astype(np.float64)
    lin_w64 = lin_w.astype(np.float64)
    lin_b64 = lin_b.astype(np.float64)
    W_eff = W_ih @ lin_w64
    b_eff = b_ih.astype(np.float64) + W_ih @ lin_b64
    W_rz = (W_eff + W_hh)[: 2 * H]
    b_rz = b_eff[: 2 * H] + b_hh.astype(np.float64)[: 2 * H]
    W_in = W_eff[2 * H :]
    b_in = b_eff[2 * H :]
    W_hn = W_hh[2 * H :]
    b_hn = b_hh.astype(np.float64)[2 * H :]
    W_big = linres_w.astype(np.float32) @ lin_w.astype(np.float32)
    b_big = linres_b.astype(np.float64) + linres_w.astype(np.float64) @ lin_b64
    return W_rz, b_rz, W_in, b_in, W_hn, b_hn, W_big, b_big


def host_step0(C, init_hidden, W_ih, W_hh, b_ih, b_hh, linC_w, linC_b):
    h0 = init_hidden[0].astype(np.float64)
    x0 = C[:, 0, :].astype(np.float64) @ linC_w.astype(np.float64).T + linC_b.astype(
        np.float64
    )
    gi = x0 @ W_ih.astype(np.float64).T + b_ih.astype(np.float64)
    gh = h0 @ W_hh.astype(np.float64).T + b_hh.astype(np.float64)
    r = 1.0 / (1.0 + np.exp(-(gi[:, :H] + gh[:, :H])))
    z = 1.0 / (1.0 + np.exp(-(gi[:, H : 2 * H] + gh[:, H : 2 * H])))
    n = np.tanh(gi[:, 2 * H :] + r * gh[:, 2 * H :])
    return (1.0 - z) * n + z * h0


def make_input_maps(inputs):
    W_rz, b_rz, W_in, b_in, W_hn, b_hn, W_big, b_big = fold_weights(
        np.asarray(inputs["W_ih"]),
        np.asarray(inputs["W_hh"]),
        np.asarray(inputs["b_ih"]),
        np.asarray(inputs["b_hh"]),
        np.asarray(inputs["lin_w"]),
        np.asarray(inputs["lin_b"]),
        np.asarray(inputs["linres_w"]),
        np.asarray(inputs["linres_b"]),
    )
    h1 = host_step0(
        np.asarray(inputs["C"]),
        np.asarray(inputs["init_hidden"]),
        np.asarray(inputs["W_ih"]),
        np.asarray(inputs["W_hh"]),
        np.asarray(inputs["b_ih"]),
        np.asarray(inputs["b_hh"]),
        np.asarray(inputs["linC_w"]),
        np.asarray(inputs["linC_b"]),
    )

    Wcat = np.concatenate([W_rz.T, W_in.T, W_hn.T], axis=1)  # [H, 4096]
    wcat = np.ascontiguousarray(Wcat.reshape(KCH, 128, 4096).astype(np.float32)).astype(
        _bf16
    )
    gbias_row = np.concatenate([b_rz, b_in, b_hn]).astype(np.float32)
    gbias = np.ascontiguousarray(
        np.broadcast_to(gbias_row[None, :], (128, 4096))
    ).astype(_bf16)
    h1_bf = h1.astype(np.float32).astype(_bf16)
    # hstate layout: [p, k*128 + b] = h1[b, k*128 + p]
    h1s = np.ascontiguousarray(
        h1_bf.astype(np.float32).T.reshape(KCH, 128, B).transpose(1, 0, 2).reshape(128, H)
    ).astype(_bf16)
    ident = np.eye(128, dtype=np.float32).astype(_bf16)

    WbT = W_big.T.astype(np.float32)  # [H, V]
    in_maps = []
    for c in range(N_CORES):
        v0 = c * VS
        v1 = min(V, v0 + VS)
        wslice = np.zeros((H, VS), np.float32)
        wslice[:, : v1 - v0] = WbT[:, v0:v1]
        wproj = np.ascontiguousarray(wslice.reshape(KCH, 128, VS)).astype(_bf16)
        bslice = np.zeros((VS,), np.float32)
        bslice[: v1 - v0] = b_big[v0:v1].astype(np.float32)
        bproj = np.ascontiguousarray(
            np.broadcast_to(bslice[None, :], (128, VS))
        ).astype(_bf16)
        in_maps.append(
            {
                "wcat": wcat,
                "gbias": gbias,
                "h1s": h1s,
                "h1b": h1_bf,
                "ident": ident,
                "wproj": wproj,
                "bproj": bproj,
            }
        )
    return in_maps


def assemble_output(results, T):
    full = np.empty((B, T, V), np.float32)
    for c in range(N_CORES):
        v0 = c * VS
        v1 = min(V, v0 + VS)
        oc = results[c]["out"].reshape(T, B, VS)[:, :, : v1 - v0]
        full[:, :, v0:v1] = oc.transpose(1, 0, 2).astype(np.float32)
    return full


_PROGRAMS = {}


def _get_program(T, n_reps=0):
    key = (T, n_reps)
    if key not in _PROGRAMS:
        _PROGRAMS[key] = build_program(T=T, n_reps=n_reps)
    return _PROGRAMS[key]


def kernel(**inputs):
    T = int(inputs["max_len"])
    in_maps = make_input_maps(inputs)
    nc = _get_program(T)
    br = run_bass_kernel_spmd(nc, in_maps, list(range(N_CORES)))
    return assemble_output(br.results, T)

